# revision 7
# baseline (speedup 1.0000x reference)
"""Causal self-attention (B=4, T=2048, C=1024, H=16) on 8 trn2 NeuronCores.

Sharding v2: batch x head-group. Core c handles batch b=c//2 and head group
g2=c%2 (8 heads = 4 head-pairs). Each core:
 - QKV projection for its 8 heads over its batch's 2048 tokens
 - attention for 4 head-pairs (causal, diagonal-sub-sliced)
 - out-projection partial [C, T] contracted over its 512 y-channels
Host sums the two partials per batch (the "all-reduce"), 8.4 MB each
(vs 33.5 MB x 8 in the pure head-parallel variant -> 3.4x less HBM traffic).

Layouts per core (partition dim first everywhere):
  xT    [C, T]           x[b] transposed on host, bf16
  q/k   [128=2h*64, hp, T]  bf16 (q transient per tq block, k persistent)
  vAB   [128 tk, hp, 16, 3, 64] bf16: [v_h0 | pad | v_h1] per tk tile
  S^T   [tk, 2, tq]      scores transposed; exp -> p bf16
  pyA = vA.T@p_h0 = [Y_h0 | rs_h0], pyB = vB.T@p_h1 = [rs_h1 | Y_h1]
  stream_shuffle (identity mask, offset APs = cross-partition copy on DVE)
  aligns the replicated row-sums with their Y halves for the normalize.
  outT  [C, T] bf16 partial, summed+transposed on host.

All matmuls bf16 (1 cyc/row at any free size) except the out-projection
(fp32r y/Wout, 1 cyc/row at N=512). Diagonal tiles only compute the
causally-live column range (N=512-128*di); exp ops sub-slice to match.
The TileScheduler reorders instructions, so the emission interleavings
below (A/B/C phase placement, deferred C(3)) shape WHAT is available to
fill PE stalls, not the exact order. ACT (exp) is the pacing engine
during attention; all movable copies sit on DVE or, for q/k, on ACT in
the otherwise ACT-idle A phases.
"""

import numpy as np
import ml_dtypes
from contextlib import ExitStack

import concourse.bass as bass
import concourse.bacc as bacc
import concourse.mybir as mybir
import concourse.tile as tile
from concourse import bass_utils
from concourse.masks import make_identity

B, T, C = 4, 2048, 1024
H, D = 16, 64
NCORES = 8
G = 4                 # head-pairs per core
NCT = C // 128        # 8 contraction tiles for projections
TQB = 512             # tq block
NJ = T // TQB         # 4
NKT = T // 128        # 16
FP32 = mybir.dt.float32
FP32R = mybir.dt.float32r
BF16 = mybir.dt.bfloat16
AF = mybir.ActivationFunctionType
SCALE = 1.0 / np.sqrt(D)

_cached = {}

CFG = {
    "ppool_bufs": 12,
    "spsum_bufs": 2,
    "qkps_bufs": 2,
    "ypool_bufs": 20,
    "mask_engine": "vector",
    "yab_bufs": 4,
    "rc_bufs": 4,
    "o_bufs": 4,
    "drainA_engine": "vector",   # pyA -> ya drain
    "drainB_engine": "vector",   # pyB -> yb drain
}


def _emit(tc, nc, xT, wq, bq, wo, padb, padk, outT, reps=1,
          ones_mask=False):
    ctx = ExitStack()
    with ctx:
        const = ctx.enter_context(tc.tile_pool(name="const", bufs=1))
        xpool = ctx.enter_context(tc.tile_pool(name="xpool", bufs=CFG.get("x_bufs", 2)))
        qpool = ctx.enter_context(tc.tile_pool(name="qpool", bufs=CFG.get("q_bufs", 2)))
        vpool = ctx.enter_context(tc.tile_pool(name="vpool", bufs=5))
        ppool = ctx.enter_context(tc.tile_pool(name="ppool", bufs=CFG["ppool_bufs"]))
        yab = ctx.enter_context(tc.tile_pool(name="yab", bufs=CFG.get("yab_bufs", 2)))
        rcpool = ctx.enter_context(tc.tile_pool(name="rcpool", bufs=CFG.get("rc_bufs", 2)))
        ypool = ctx.enter_context(tc.tile_pool(name="ypool", bufs=CFG["ypool_bufs"]))
        opool = ctx.enter_context(tc.tile_pool(name="opool", bufs=CFG.get("o_bufs", 2)))
        spsum = ctx.enter_context(
            tc.tile_pool(name="spsum", bufs=CFG["spsum_bufs"], space="PSUM"))
        accps = ctx.enter_context(tc.tile_pool(name="accps", bufs=1, space="PSUM"))
        qkps = ctx.enter_context(
            tc.tile_pool(name="qkps", bufs=CFG["qkps_bufs"], space="PSUM"))

        # ---- constants ----
        # weights. w block DMAs are deferred into the first A(j) pass so the
        # first projection matmuls only wait on their own block, not the full
        # 6.3 MB load; DMAs rotate across engine queues for parallelism.
        dma_engines = [nc.sync, nc.scalar, nc.gpsimd]
        b_sb = const.tile([128, 3 * G], FP32)
        nc.gpsimd.dma_start(b_sb, bq)
        # weights: coarse per-ct DMAs on the scalar/gpsimd queues; the sync
        # queue is left free so A(0)'s x tiles land immediately.
        w_sb = const.tile([128, NCT, 3 * G * 128], BF16)
        for ct in range(NCT):
            (nc.scalar if ct % 2 == 0 else nc.gpsimd).dma_start(
                w_sb[:, ct, :], wq[ct * 128:(ct + 1) * 128, :])
        # k and vAB persist across the rep (rewritten per rep)
        k_sb = const.tile([128, G, T], BF16)
        vAB = const.tile([128, G, NKT, 3, 64], BF16)
        if ones_mask:
            padb_sb = None
            nc.gpsimd.memset(vAB[:, :, :, 1, :], 1.0)
        else:
            padb_sb = const.tile([128, T], BF16)
            nc.sync.dma_start(padb_sb, padb)
            for hp in range(G):
                nc.scalar.dma_start(vAB[:, hp, :, 1, :],
                                    padk.rearrange("p (i f) -> p i f", f=64))
        wo_sb = const.tile([128, G, C], FP32R)

        # gpsimd const builds go after the DMA kickoffs so they don't delay
        # the gpsimd DMA queue at startup
        ident = const.tile([128, 128], BF16)
        make_identity(nc, ident)
        tri = const.tile([128, 2, 128], BF16)
        nc.gpsimd.memset(tri, 1.0)
        for h in range(2):
            nc.gpsimd.affine_select(
                out=tri[:, h, :], in_=tri[:, h, :],
                compare_op=mybir.AluOpType.is_ge, fill=0.0,
                base=0, pattern=[[1, 128]], channel_multiplier=-1)

        def load_wo():
            for hp in range(G):
                for ot in range(0, NCT, 2):
                    dma_engines[(hp + ot) % len(dma_engines)].dma_start(
                        wo_sb[:, hp, ot * 128:(ot + 2) * 128],
                        wo[hp * 128:(hp + 1) * 128,
                           ot * 128:(ot + 2) * 128])

        pending_c3 = [None]

        for rep in range(reps):
            q_tiles = {}
            y_tiles = {}

            def emit_A(j):
                x = xpool.tile([128, NCT, TQB], BF16, name=f"{rep}_x_{j}",
                               tag="x")
                for ct in range(NCT):
                    nc.sync.dma_start(
                        x[:, ct, :],
                        xT[ct * 128:(ct + 1) * 128, j * TQB:(j + 1) * TQB])
                q = qpool.tile([128, G, TQB], BF16, name=f"{rep}_q_{j}", tag="q")
                q_tiles[j] = q
                v_tiles = {}
                for hp in range(G):
                    for ft in range(3):
                        ps = qkps.tile([128, TQB], FP32,
                                       name=f"{rep}_ps_{j}_{hp}_{ft}", tag="ps")
                        for ct in range(NCT):
                            nc.tensor.matmul(
                                ps,
                                lhsT=w_sb[:, ct,
                                          (hp * 3 + ft) * 128:
                                          (hp * 3 + ft + 1) * 128],
                                rhs=x[:, ct, :],
                                start=(ct == 0), stop=(ct == NCT - 1))
                        col = hp * 3 + ft
                        # bias-adds on DVE: ACT is the binding engine during
                        # attention (exp), DVE has slack
                        if ft == 0:
                            nc.scalar.activation(q[:, hp, :], ps, AF.Identity,
                                                 scale=float(SCALE),
                                                 bias=b_sb[:, col:col + 1])
                        elif ft == 1:
                            nc.scalar.activation(
                                k_sb[:, hp, j * TQB:(j + 1) * TQB], ps,
                                AF.Identity, bias=b_sb[:, col:col + 1])
                        else:
                            v = vpool.tile([128, TQB], BF16,
                                           name=f"{rep}_v_{j}_{hp}", tag="v")
                            nc.vector.tensor_scalar_add(
                                v, ps, b_sb[:, col:col + 1])
                            if not ones_mask:
                                nc.vector.tensor_mul(
                                    v, v, padb_sb[:, j * TQB:(j + 1) * TQB])
                            v_tiles[hp] = v
                # transposes go after all projections: in-order PE would
                # otherwise stall on the ACT/DVE v pipeline mid-phase
                for hp in range(G):
                    v = v_tiles[hp]
                    pb = qkps.tile([128, 4, 2, 64], BF16,
                                   name=f"{rep}_pb_{j}_{hp}", tag="ps")
                    for t in range(4):
                        nc.tensor.transpose(
                            pb[:, t, :, :], v[:, t * 128:(t + 1) * 128], ident)
                    for h in range(2):
                        nc.vector.tensor_copy(
                            vAB[:, hp, j * 4:(j + 1) * 4, 2 * h, :],
                            pb[:, :, h, :])

            def emit_B(j, hp):
                q = q_tiles[j]
                pyA = accps.tile([128, TQB], FP32, name=f"{rep}_pyA_{j}_{hp}",
                                 tag="pyA")
                pyB = accps.tile([128, TQB], FP32, name=f"{rep}_pyB_{j}_{hp}",
                                 tag="pyB")
                ntk = 4 * (j + 1)
                # interleave the 4 diagonal tiles (small-N matmuls but
                # full-size exp -> ACT-heavy) among the full tiles so the
                # ACT engine never falls behind PE in a burst. First element
                # must be a lo=0 tile (PV start=True zeroes the full range).
                fulls = list(range(4 * j))
                diags = [4 * j + d for d in range(4)]
                if j == 0:
                    seq = diags
                else:
                    seq, k0 = [], 0
                    for d in diags:
                        seq += fulls[k0:k0 + j]
                        seq.append(d)
                        k0 += j
                    seq += fulls[k0:]

                def emit_S(i):
                    di = i - 4 * j
                    lo = 128 * di if di > 0 else 0
                    s2 = spsum.tile([128, 2, TQB], FP32,
                                    name=f"{rep}_s_{j}_{hp}_{i}", tag="s")
                    for h in range(2):
                        nc.tensor.matmul(
                            s2[:, h, lo:TQB],
                            lhsT=k_sb[h * 64:(h + 1) * 64, hp,
                                      i * 128:(i + 1) * 128],
                            rhs=q[h * 64:(h + 1) * 64, hp, lo:TQB],
                            start=True, stop=True,
                            tile_position=(h * 64, 0))
                    p = ppool.tile([128, 2, TQB], BF16,
                                   name=f"{rep}_p_{j}_{hp}_{i}", tag="p")
                    nc.scalar.activation(p[:, :, lo:TQB], s2[:, :, lo:TQB],
                                         AF.Exp)
                    if di >= 0:
                        eng = (nc.gpsimd if CFG["mask_engine"] == "pool"
                               else nc.vector)
                        eng.tensor_mul(p[:, :, lo:lo + 128],
                                       p[:, :, lo:lo + 128], tri)
                    return p, lo

                def emit_PV(i, p, lo, first, last):
                    nc.tensor.matmul(pyA[:, lo:TQB],
                                     lhsT=vAB[:, hp, i, 0:2, :],
                                     rhs=p[:, 0, lo:TQB],
                                     start=first, stop=last)
                    nc.tensor.matmul(pyB[:, lo:TQB],
                                     lhsT=vAB[:, hp, i, 1:3, :],
                                     rhs=p[:, 1, lo:TQB],
                                     start=first, stop=last)

                # software skew: S one step ahead of PV so PV never waits exp
                pending = None
                for n, i in enumerate(seq):
                    p, lo = emit_S(i)
                    if pending is not None:
                        emit_PV(*pending, first=(n == 1), last=False)
                    pending = (i, p, lo)
                emit_PV(*pending, first=(ntk == 1), last=True)
                # drain PSUM accumulators (frees banks for the next unit)
                ya = yab.tile([128, TQB], BF16, name=f"{rep}_ya_{j}_{hp}",
                              tag="ya")
                yb = yab.tile([128, TQB], BF16, name=f"{rep}_yb_{j}_{hp}",
                              tag="yb")
                if CFG["drainA_engine"] == "vector":
                    nc.vector.tensor_copy(ya, pyA)
                else:
                    nc.scalar.activation(ya, pyA, AF.Copy)
                if CFG["drainB_engine"] == "scalar":
                    nc.scalar.activation(yb, pyB, AF.Copy)
                else:
                    nc.vector.tensor_copy(yb, pyB)
                # align the row-sums with their Y halves: stream_shuffle
                # with an identity mask and offset APs is a cross-partition
                # copy on DVE (no PE, no PSUM slot)
                rs_al = rcpool.tile([128, TQB], BF16,
                                    name=f"{rep}_rs_{j}_{hp}", tag="rs")
                idmask = list(range(32))
                nc.vector.stream_shuffle(rs_al[0:64, :], ya[64:128, :], idmask)
                nc.vector.stream_shuffle(rs_al[64:128, :], yb[0:64, :], idmask)
                recip = rcpool.tile([128, TQB], FP32, name=f"{rep}_rc_{j}_{hp}",
                                    tag="rc")
                nc.vector.reciprocal(recip, rs_al)
                y = ypool.tile([128, TQB], FP32R, name=f"{rep}_y_{j}_{hp}",
                               tag="y")
                y_tiles[(j, hp)] = y
                nc.vector.tensor_mul(y[0:64, :], ya[0:64, :], recip[0:64, :])
                nc.vector.tensor_mul(y[64:128, :], yb[64:128, :],
                                     recip[64:128, :])

            def emit_C(j, ots=None, yt=None):
                if yt is None:
                    yt = y_tiles
                for ot in (range(NCT) if ots is None else ots):
                    po = qkps.tile([128, TQB], FP32, name=f"{rep}_po_{j}_{ot}",
                                   tag="ps")
                    for hp in range(G):
                        nc.tensor.matmul(
                            po, lhsT=wo_sb[:, hp, ot * 128:(ot + 1) * 128],
                            rhs=yt[(j, hp)],
                            start=(hp == 0), stop=(hp == G - 1))
                    o = opool.tile([128, TQB], BF16, name=f"{rep}_o_{j}_{ot}",
                                   tag="o")
                    if j == NJ - 1 and ot % 2 == 1:
                        nc.scalar.activation(o, po, AF.Copy)
                    else:
                        nc.vector.tensor_copy(o, po)
                    nc.sync.dma_start(
                        outT[ot * 128:(ot + 1) * 128,
                             j * TQB:(j + 1) * TQB], o)

            emit_A(0)
            if pending_c3[0] is not None:
                pending_c3[0]()
                pending_c3[0] = None
            emit_A(1)
            if rep == 0:
                load_wo()
            for hp in range(G):
                emit_B(0, hp)
            emit_A(2)
            # interleave out-projections (pure PE) between B units as filler
            # for the exp-throughput deficit of the attention stretches;
            # singles maximize slot-rotation interleaving with B-phase PSUM
            for hp in range(G):
                emit_B(1, hp)
                emit_C(0, [hp])
            emit_A(3)
            for hp in range(G):
                emit_B(2, hp)
                emit_C(0, [4 + hp])
                emit_C(1, [2 * hp])
            for hp in range(G):
                emit_B(3, hp)
                emit_C(1, [2 * hp + 1])
                emit_C(2, [2 * hp, 2 * hp + 1])
            # the last block's out-projection is deferred into the next rep's
            # A(0) window (pure-PE filler there); emitted directly on the
            # final rep
            if rep == reps - 1:
                emit_C(3)
            else:
                pending_c3[0] = (
                    lambda fn=emit_C, yt=y_tiles: fn(3, None, yt))



def build(reps=1, ones_mask=False):
    nc = bacc.Bacc()
    xT = nc.dram_tensor("xT", [C, T], BF16, kind="ExternalInput")
    wq = nc.dram_tensor("wq", [C, 3 * G * 128], BF16, kind="ExternalInput")
    bq = nc.dram_tensor("bq", [128, 3 * G], FP32, kind="ExternalInput")
    wo = nc.dram_tensor("wo", [G * 128, C], FP32R, kind="ExternalInput")
    padb = nc.dram_tensor("padb", [128, T], BF16, kind="ExternalInput")
    padk = nc.dram_tensor("padk", [128, NKT * 64], BF16, kind="ExternalInput")
    outT = nc.dram_tensor("outT", [C, T], BF16, kind="ExternalOutput")
    with tile.TileContext(nc) as tc:
        _emit(tc, nc, xT.ap(), wq.ap(), bq.ap(), wo.ap(), padb.ap(),
              padk.ap(), outT.ap(), reps=reps, ones_mask=ones_mask)
    nc.compile()
    return nc


def make_in_maps(x, attention_mask, Wqkv, bqkv, Wout):
    bf = ml_dtypes.bfloat16
    in_maps = []
    xTb, padbb, padkb = [], [], []
    for b in range(B):
        xTb.append(np.ascontiguousarray(x[b].T).astype(bf))
        pad = attention_mask[b].astype(np.float32)
        padbb.append(np.ascontiguousarray(
            np.broadcast_to(pad[None, :], (128, T))).astype(bf))
        pk = np.broadcast_to(
            pad.reshape(16, 128, 1), (16, 128, 64))
        padkb.append(np.ascontiguousarray(
            pk.transpose(1, 0, 2).reshape(128, 16 * 64)).astype(bf))
    for c in range(NCORES):
        b, g2 = c // 2, c % 2
        wq_blocks, bq_blocks, wo_blocks = [], [], []
        for hp in range(G):
            h0 = 8 * g2 + 2 * hp
            rows2 = np.r_[64 * h0:64 * h0 + 128]
            for ft in range(3):
                rows = ft * C + rows2
                wq_blocks.append(Wqkv[rows, :])
                bq_blocks.append(bqkv[rows] * (0.125 if ft == 0 else 1.0))
            wo_blocks.append(Wout[:, rows2].T)
        wq_c = np.ascontiguousarray(
            np.concatenate(wq_blocks, 0).T).astype(bf)
        bq_c = np.ascontiguousarray(
            np.stack(bq_blocks, 1).astype(np.float32, copy=False))
        wo_c = np.ascontiguousarray(
            np.concatenate(wo_blocks, 0).astype(np.float32, copy=False))
        in_maps.append({"xT": xTb[b], "wq": wq_c, "bq": bq_c, "wo": wo_c,
                       "padb": padbb[b], "padk": padkb[b]})
    return in_maps


def kernel(x, attention_mask, Wqkv, bqkv, Wout, _trace=False):
    x = np.asarray(x)
    attention_mask = np.asarray(attention_mask)
    Wqkv = np.asarray(Wqkv)
    bqkv = np.asarray(bqkv)
    Wout = np.asarray(Wout)
    ones = bool(np.all(attention_mask == 1))
    key = ("nc_ones" if ones else "nc")
    if key not in _cached:
        _cached[key] = build(ones_mask=ones)
    nc = _cached[key]
    in_maps = make_in_maps(x, attention_mask, Wqkv, bqkv, Wout)
    res = bass_utils.run_bass_kernel_spmd(
        nc, in_maps, core_ids=list(range(NCORES)), trace=_trace)
    out = np.empty((B, T, C), np.float32)
    for b in range(B):
        acc = res.results[2 * b]["outT"].astype(np.float32)
        acc += res.results[2 * b + 1]["outT"].astype(np.float32)
        out[b] = acc.T
    if _trace:
        _cached["last_result"] = res
    return out
